# revision 1
# baseline (speedup 1.0000x reference)
import numpy as np
import jax
import jax.numpy as jnp

# Problem constants (hardcoded per contract -- kernel.py is self-contained).
B, V, H, W, CIN, COUT = 2, 4, 256, 256, 64, 64
WINDOW = 1
GAMMA = 0.1
EPS = 1e-8
HW = H * W
NDEV = 8

# Sharding: one (batch, view) pair per NeuronCore -- B*V == 8 == n_cores.
# The cross-view gather couples only views within a window of 1, so each
# core receives its own view's features plus (up to) two neighbor views'
# features, replicated at input-distribution time; conv weights replicated.


def _pair_fn(feats_self, feats_nb, w2c3_nb, intr_nb, flags, ph, w1, b1, w2, b2):
    # feats_self (hw,c); feats_nb (2,hw,c); w2c3_nb (2,3,4); intr_nb (2,3,3)
    # flags (2,); ph (hw,4) homogeneous points of THIS view.
    acc = feats_self
    norm = jnp.float32(1.0)
    for n in range(2):
        cam = ph @ w2c3_nb[n].T                      # (hw,3) camera coords
        z = cam[:, 2]
        ndc = (cam / (z + EPS)[:, None]) @ intr_nb[n].T
        u, v = ndc[:, 0], ndc[:, 1]
        px = jnp.floor(u * W).astype(jnp.int32)
        py = jnp.floor(v * H).astype(jnp.int32)
        mask = (u >= 0) & (u < 1) & (v >= 0) & (v < 1) & (z > EPS)
        idx = jnp.clip(py, 0, H - 1) * W + jnp.clip(px, 0, W - 1)
        gathered = jnp.take(feats_nb[n], idx, axis=0)
        upd = jnp.where(mask[:, None], gathered, jnp.float32(0.0))
        scale = flags[n] * GAMMA * mask.sum().astype(jnp.float32) / (H * W)
        acc = acc + upd * scale
        norm = norm + scale
    out = acc / norm
    img = out.reshape(H, W, CIN).transpose(2, 0, 1)[None]   # (1,c,h,w)
    x = jax.lax.conv_general_dilated(
        img, w1, (1, 1), 'SAME', dimension_numbers=('NCHW', 'OIHW', 'NCHW'))
    x = x + b1[None, :, None, None]
    x = jax.nn.gelu(x, approximate=False)
    x = jax.lax.conv_general_dilated(
        x, w2, (1, 1), 'SAME', dimension_numbers=('NCHW', 'OIHW', 'NCHW'))
    x = x + b2[None, :, None, None]
    return x[0].transpose(1, 2, 0)                          # (h,w,cout)


_pmapped = jax.pmap(_pair_fn)


def kernel(means, depths, gs_feats, intrinsics, extrinsics, w1, b1, w2, b2):
    means = np.asarray(means, np.float32)
    gs_feats = np.asarray(gs_feats, np.float32)
    intrinsics = np.asarray(intrinsics, np.float32)
    extrinsics = np.asarray(extrinsics, np.float32)
    w1 = np.asarray(w1, np.float32); b1 = np.asarray(b1, np.float32)
    w2 = np.asarray(w2, np.float32); b2 = np.asarray(b2, np.float32)

    w2c = np.linalg.inv(extrinsics.astype(np.float32)).astype(np.float32)  # (B,V,4,4)
    feats_flat = gs_feats.reshape(B, V, HW, CIN)
    pts = means.reshape(B, V, HW, 3)

    fs, fn, m_nb, i_nb, fl, phs = [], [], [], [], [], []
    for b in range(B):
        for j in range(V):
            fs.append(feats_flat[b, j])
            ph = np.concatenate([pts[b, j], np.ones((HW, 1), np.float32)], axis=1)
            phs.append(ph)
            nb_f, nb_m, nb_i, nb_fl = [], [], [], []
            for k in (j - 1, j + 1):
                if 0 <= k < V:
                    nb_f.append(feats_flat[b, k])
                    nb_m.append(w2c[b, k, :3, :])
                    nb_i.append(intrinsics[b, k])
                    nb_fl.append(1.0)
                else:
                    nb_f.append(feats_flat[b, j])
                    nb_m.append(w2c[b, j, :3, :])
                    nb_i.append(intrinsics[b, j])
                    nb_fl.append(0.0)
            fn.append(np.stack(nb_f)); m_nb.append(np.stack(nb_m))
            i_nb.append(np.stack(nb_i)); fl.append(np.array(nb_fl, np.float32))

    out = _pmapped(
        np.stack(fs), np.stack(fn), np.stack(m_nb), np.stack(i_nb),
        np.stack(fl), np.stack(phs),
        np.broadcast_to(w1, (NDEV,) + w1.shape),
        np.broadcast_to(b1, (NDEV,) + b1.shape),
        np.broadcast_to(w2, (NDEV,) + w2.shape),
        np.broadcast_to(b2, (NDEV,) + b2.shape),
    )
    return np.asarray(out).reshape(B, V, H, W, COUT).astype(np.float32)
